# revision 2
# baseline (speedup 1.0000x reference)
"""GCN 2-layer Bass kernel for TRN2, sharded over NCORES cores.

Sharding: nodes split evenly across cores; edges partitioned by destination
node; weights replicated; layer-2 source features exchanged via AllGather.

Math (per reference):
    h   = relu(Ahat @ (x @ W1) + b1)    = relu((Ahat @ x) @ W1 + b1)
    out = Ahat @ (h @ W2) + b2
where Ahat = D^-1/2 (A+I) D^-1/2 on the self-loop-augmented graph.

Factorization used on device: with x' = dinv*x (host-prescaled),
    Ahat x = dinv_dst * ((A+I) x')
so gathers read pre-scaled rows, selection matrices are pure 0/1
(is_equal, padding = -1 never matches), and the dst-side dinv is applied
where nodes sit on PSUM partitions.

All per-edge tables are bf16 with 128-element rows (cols 0..63 = payload,
64..127 = don't-care) so each dma_gather element is the 256B minimum while
the selection matmuls run at bf16 rate (1 cycle/row vs 4 for fp32r at
moving-dim 128).

Device algorithm per core (owns NLOC nodes, NB blocks of BN=128 dst nodes):
  phase A: per group of GB blocks: dma_gather x' rows for the group's edges
      (one gather per 25088-row source chunk so indices fit int16, spread
      over 4 SWDGE queues). Per block: identity matmul for self-loops
      (x'_loc block via plain DMA) + per 128-edge chunk a PE matmul with a
      0/1 selection matrix (built 8 chunks per DVE is_equal), accumulating
      q.T [64, BN] in PSUM. Then inline node-major tail:
      z = (q.T).T @ W1; h = relu(dinv*z + b1); hT = h.T (PE);
      t2 = hT.T @ W2; t2loc row-block = dinv*t2 (ACT scale).
  phase B: AllGather t2loc -> t2full [NPAD, 128] bf16.
  phase C: same gather structure over t2full, but with the selection
      matrix as the stationary operand (out node-major [BN, 64] directly,
      no PE transpose in the tail); self-loops use identity stationary;
      out_block = dinv*psum + b2.
"""

import sys

sys.path.insert(0, "/opt/trn_rl_repo")

import ml_dtypes
import numpy as np

import concourse.bass as bass
import concourse.mybir as mybir
import concourse.tile as tile
from concourse import bacc

F32 = mybir.dt.float32
BF16 = mybir.dt.bfloat16
I16 = mybir.dt.int16
AF = mybir.ActivationFunctionType
ALU = mybir.AluOpType
NPBF16 = ml_dtypes.bfloat16

NCHUNK = 4  # source-table chunks (int16 index range)
NQ = 4  # SWDGE queues
FW = 128  # gather element width (bf16 elems; 256B = SWDGE minimum)


def build_gcn_nc(cfg, layout):
    NPAD, NLOCP, NB, BN = cfg["NPAD"], cfg["NLOCP"], cfg["NB"], cfg["BN"]
    F, H, C, NCORES = cfg["F"], cfg["H"], cfg["C"], cfg["NCORES"]
    CH = NPAD // NCHUNK
    G = layout["G_cols"]
    groups = layout["groups"]
    OHSLAB = cfg.get("OHSLAB", 8)
    CP = cfg["CP"]  # C padded (t2 row payload width)

    nc = bacc.Bacc(
        "TRN2",
        target_bir_lowering=False,
        debug=False,
        num_devices=NCORES,
        num_swdge_queues=NQ,
    )

    # ---------------- I/O ----------------
    x_d = nc.dram_tensor("x_pad", [NPAD, FW], BF16, kind="ExternalInput")
    xloc_d = nc.dram_tensor("x_loc", [NLOCP, FW], BF16, kind="ExternalInput")
    idxg_d = nc.dram_tensor("idxg", [128, 8 * G], I16, kind="ExternalInput")
    dstlocg_d = nc.dram_tensor("dstlocg", [128, G], BF16, kind="ExternalInput")
    dinvb_d = nc.dram_tensor("dinvb", [128, NB], F32, kind="ExternalInput")
    w1_d = nc.dram_tensor("W1", [F, H], BF16, kind="ExternalInput")
    b1rep_d = nc.dram_tensor("b1rep", [128, H], F32, kind="ExternalInput")
    w2_d = nc.dram_tensor("W2", [H, CP], BF16, kind="ExternalInput")
    b2rep_d = nc.dram_tensor("b2rep", [128, C], F32, kind="ExternalInput")
    iota_d = nc.dram_tensor("iota", [128, OHSLAB * BN], BF16, kind="ExternalInput")
    ident_d = nc.dram_tensor("ident", [128, 128], BF16, kind="ExternalInput")
    out_d = nc.dram_tensor("out", [NLOCP, C], F32, kind="ExternalOutput")

    qctr = [0]

    def next_q():
        q = qctr[0] % NQ
        qctr[0] += 1
        return q

    with tile.TileContext(nc) as tc:
        with (
            tc.tile_pool(name="const", bufs=1) as cstp,
            tc.tile_pool(name="dram", bufs=1, space="DRAM") as dram_pool,
            tc.tile_pool(name="gat", bufs=cfg.get("GBUFS", 2)) as gpool,
            tc.tile_pool(name="ohb", bufs=cfg.get("OHBBUFS", 4)) as ohbpool,
            tc.tile_pool(name="xl", bufs=4) as xlpool,
            tc.tile_pool(name="cp", bufs=4) as cpool,
            tc.tile_pool(name="ps_pT", bufs=2, space="PSUM") as ps_pT,
            tc.tile_pool(name="ps_h", bufs=2, space="PSUM") as ps_h,
            tc.tile_pool(name="ps_t2", bufs=2, space="PSUM") as ps_t2,
            tc.tile_pool(name="ps_tr", bufs=2, space="PSUM") as ps_tr,
        ):
            idxg_s = cstp.tile([128, 8 * G], I16, name="idxg_s")
            dstlocg_s = cstp.tile([128, G], BF16, name="dstlocg_s")
            dinvb_s = cstp.tile([128, NB], F32, name="dinvb_s")
            w1_s = cstp.tile([F, H], BF16, name="w1_s")
            b1rep_s = cstp.tile([128, H], F32, name="b1rep_s")
            w2_s = cstp.tile([H, CP], BF16, name="w2_s")
            b2rep_s = cstp.tile([128, C], F32, name="b2rep_s")
            iota_s = cstp.tile([128, OHSLAB * BN], BF16, name="iota_s")
            ident_s = cstp.tile([128, 128], BF16, name="ident_s")
            t2stage = cstp.tile([128, NB * FW], BF16, name="t2stage")
            outstage = cstp.tile([128, NB * C], F32, name="outstage")

            nc.sync.dma_start(out=idxg_s[:], in_=idxg_d[:])
            nc.sync.dma_start(out=dstlocg_s[:], in_=dstlocg_d[:])
            nc.sync.dma_start(out=dinvb_s[:], in_=dinvb_d[:])
            nc.sync.dma_start(out=w1_s[:], in_=w1_d[:])
            nc.sync.dma_start(out=b1rep_s[:], in_=b1rep_d[:])
            nc.sync.dma_start(out=w2_s[:], in_=w2_d[:])
            nc.sync.dma_start(out=b2rep_s[:], in_=b2rep_d[:])
            nc.sync.dma_start(out=iota_s[:], in_=iota_d[:])
            nc.sync.dma_start(out=ident_s[:], in_=ident_d[:])
            # t2 rows are FW-padded; zero the pad columns once
            # (memset rejects 16-bit dtypes in walrus codegen paths; f32 view)
            nc.vector.memset(t2stage[:].bitcast(F32), 0.0)

            t2loc = dram_pool.tile([NLOCP, FW], BF16, name="t2loc")
            t2full = dram_pool.tile(
                [NPAD, FW], BF16, name="t2full", addr_space="Shared"
            )

            def layer_pass(lay, src_t, srcloc_t):
                PW = F if lay == 0 else CP  # payload width of source rows
                for grp in groups:
                    g0 = grp["col0"]
                    gcols = grp["ncols"]
                    if gcols > 0:
                        xg = gpool.tile([128, gcols * FW], BF16, tag="xg")
                    for s in range(NCHUNK):
                        k_gs = grp["k_gs"][s]
                        if k_gs == 0:
                            continue
                        c0 = grp["s_col0"][s]
                        n = 128 * k_gs
                        nc.gpsimd.dma_gather(
                            out_ap=xg[
                                :, (c0 - g0) * FW : (c0 - g0 + k_gs) * FW
                            ].rearrange("p (c f) -> p c f", f=FW),
                            in_ap=src_t[s * CH : (s + 1) * CH, :],
                            idxs_ap=idxg_s[:, 8 * c0 : 8 * (c0 + k_gs)],
                            num_idxs=n,
                            num_idxs_reg=n,
                            elem_size=FW,
                            single_packet=(n <= 1024),
                            queue_num=next_q(),
                        )
                    for blk in grp["blocks"]:
                        b = blk["b"]
                        cols = blk["cols"]
                        ncols = len(cols)
                        # self-loops: local block with identity weights
                        # (both dinv factors live in the pre/post scaling)
                        xlb = xlpool.tile([128, FW], BF16, tag="xlb")
                        nc.sync.dma_start(
                            out=xlb[:],
                            in_=srcloc_t[b * BN : (b + 1) * BN, :],
                        )
                        if lay == 0:
                            pT = ps_pT.tile([F, BN], F32, tag="pT")
                            nc.tensor.matmul(
                                pT[:],
                                lhsT=xlb[:, :F],
                                rhs=ident_s[:, :BN],
                                start=True,
                                stop=(ncols == 0),
                            )
                        else:
                            o_ps = ps_pT.tile([BN, CP], F32, tag="pT")
                            nc.tensor.matmul(
                                o_ps[:],
                                lhsT=ident_s[:, :BN],
                                rhs=xlb[:, :CP],
                                start=True,
                                stop=(ncols == 0),
                            )
                        # 0/1 selection matrices, OHSLAB chunks per DVE op
                        bc0 = blk["bcol0"]
                        for sl0 in range(0, ncols, OHSLAB):
                            slw = min(OHSLAB, ncols - sl0)
                            ohb = ohbpool.tile(
                                [128, OHSLAB * BN], BF16, tag="ohb"
                            )
                            nc.vector.tensor_tensor(
                                out=ohb[:, : slw * BN].rearrange(
                                    "p (k n) -> p k n", n=BN
                                ),
                                in0=iota_s[:, : slw * BN].rearrange(
                                    "p (k n) -> p k n", n=BN
                                ),
                                in1=dstlocg_s[
                                    :, bc0 + sl0 : bc0 + sl0 + slw, None
                                ].to_broadcast([128, slw, BN]),
                                op=ALU.is_equal,
                            )
                            for i in range(slw):
                                col = cols[sl0 + i]
                                last = sl0 + i == ncols - 1
                                if lay == 0:
                                    nc.tensor.matmul(
                                        pT[:],
                                        lhsT=xg[
                                            :, col * FW - g0 * FW : col * FW - g0 * FW + F
                                        ],
                                        rhs=ohb[:, i * BN : (i + 1) * BN],
                                        start=False,
                                        stop=last,
                                    )
                                else:
                                    nc.tensor.matmul(
                                        o_ps[:],
                                        lhsT=ohb[:, i * BN : (i + 1) * BN],
                                        rhs=xg[
                                            :, col * FW - g0 * FW : col * FW - g0 * FW + CP
                                        ],
                                        start=False,
                                        stop=last,
                                    )
                        if lay == 0:
                            # inline node-major dense tail for this block
                            qsb = cpool.tile([F, BN], BF16, tag="qsb")
                            nc.scalar.copy(out=qsb[:], in_=pT[:])
                            z_ps = ps_h.tile([BN, H], F32, tag="z")
                            nc.tensor.matmul(
                                z_ps[:],
                                lhsT=qsb[:],
                                rhs=w1_s[:],
                                start=True,
                                stop=True,
                            )
                            h_sb = cpool.tile([BN, H], F32, tag="h")
                            nc.scalar.activation(
                                out=h_sb[:],
                                in_=z_ps[:],
                                func=AF.Copy,
                                bias=0.0,
                                scale=dinvb_s[:, b : b + 1],
                            )
                            h2_sb = cpool.tile([BN, H], F32, tag="h2")
                            nc.vector.tensor_tensor(
                                out=h2_sb[:],
                                in0=h_sb[:],
                                in1=b1rep_s[:],
                                op=ALU.add,
                            )
                            hr_sb = cpool.tile([BN, H], BF16, tag="hr")
                            nc.scalar.activation(
                                out=hr_sb[:],
                                in_=h2_sb[:],
                                func=AF.Relu,
                                bias=0.0,
                                scale=1.0,
                            )
                            hT_ps = ps_tr.tile([H, BN], BF16, tag="tr")
                            nc.tensor.matmul(
                                hT_ps[:],
                                lhsT=hr_sb[:],
                                rhs=ident_s[:, :BN],
                                is_transpose=True,
                                start=True,
                                stop=True,
                            )
                            hT_sb = cpool.tile([H, BN], BF16, tag="hTs")
                            nc.scalar.copy(out=hT_sb[:], in_=hT_ps[:])
                            t2_ps = ps_t2.tile([BN, CP], F32, tag="t2")
                            nc.tensor.matmul(
                                t2_ps[:],
                                lhsT=hT_sb[:],
                                rhs=w2_s[:],
                                start=True,
                                stop=True,
                            )
                            # t2 row pre-scaled by dinv (layer-2 src side)
                            nc.scalar.activation(
                                out=t2stage[:, b * FW : b * FW + CP],
                                in_=t2_ps[:],
                                func=AF.Copy,
                                bias=0.0,
                                scale=dinvb_s[:, b : b + 1],
                            )
                        else:
                            y = cpool.tile([BN, C], F32, tag="y")
                            nc.scalar.activation(
                                out=y[:],
                                in_=o_ps[:, :C],
                                func=AF.Copy,
                                bias=0.0,
                                scale=dinvb_s[:, b : b + 1],
                            )
                            nc.vector.tensor_tensor(
                                out=outstage[:, b * C : (b + 1) * C],
                                in0=y[:],
                                in1=b2rep_s[:],
                                op=ALU.add,
                            )

            # ---------------- phase A ----------------
            phases = cfg.get("PHASES", "A,B,C").split(",")
            if "A" in phases:
                layer_pass(0, x_d, xloc_d)
                nc.sync.dma_start(
                    out=t2loc.rearrange("(b p) c -> p b c", p=128),
                    in_=t2stage[:].rearrange("p (b c) -> p b c", b=NB),
                )

            # ---------------- phase B: AllGather ----------------
            if "B" in phases:
                if NCORES > 1:
                    nc.gpsimd.collective_compute(
                        "AllGather",
                        ALU.bypass,
                        replica_groups=[list(range(NCORES))],
                        ins=[t2loc[:, :]],
                        outs=[t2full[:, :]],
                    )
                else:
                    nc.sync.dma_start(out=t2full[:, :], in_=t2loc[:, :])

            # ---------------- phase C ----------------
            if "C" in phases:
                layer_pass(1, t2full, t2loc)
                nc.sync.dma_start(
                    out=out_d.rearrange("(b p) c -> p b c", p=128),
                    in_=outstage[:].rearrange("p (b c) -> p b c", b=NB),
                )
            else:
                nc.sync.dma_start(
                    out=out_d[:, :], in_=t2loc[:, :C].bitcast(F32)[:, :C]
                )

    nc.compile()
    return nc


# ====================== host-side preprocessing ======================


def prep(x, edge_index, W1, b1, W2, b2, NCORES=8, BN=128, GB=8, OHSLAB=8):
    """Partition/pad inputs. Returns (cfg, layout, in_maps)."""
    N, F = x.shape
    H = W1.shape[1]
    C = W2.shape[1]
    CP = 64  # C padded to 64 (t2 payload width)
    assert N % NCORES == 0
    NLOC = N // NCORES
    NB = -(-NLOC // BN)
    NLOCP = NB * BN
    NPAD = NCORES * NLOCP
    assert NPAD % NCHUNK == 0
    CH = NPAD // NCHUNK
    assert CH <= 32768, "chunk exceeds int16 index range"

    src = np.asarray(edge_index[0], dtype=np.int64)
    dst = np.asarray(edge_index[1], dtype=np.int64)

    deg = np.bincount(dst, minlength=N).astype(np.float64) + 1.0
    dinv = (1.0 / np.sqrt(deg)).astype(np.float32)

    # pre-scale x by dinv (source-side factor of Ahat)
    xs = np.asarray(x, dtype=np.float32) * dinv[:, None]
    x_pad = np.zeros((NPAD, FW), dtype=NPBF16)
    xv = x_pad.reshape(NCORES, NLOCP, FW)
    xv[:, :NLOC, :F] = xs.reshape(NCORES, NLOC, F).astype(NPBF16)
    src_pad = src + (NLOCP - NLOC) * (src // NLOC)

    core = dst // NLOC
    dstloc = dst - core * NLOC
    blk = dstloc // BN
    within = (dstloc % BN).astype(np.float32)
    schunk = src_pad // CH

    key = ((core * NB + blk) * NCHUNK + schunk).astype(np.int64)
    order = np.argsort(key, kind="stable")
    key_o = key[order]
    src_o = src_pad[order]
    within_o = within[order]

    counts = np.bincount(key_o, minlength=NCORES * NB * NCHUNK).reshape(
        NCORES, NB, NCHUNK
    )
    k_bs = -(-counts.max(axis=0) // 128)  # [NB, NCHUNK] uniform across cores

    ngroups = -(-NB // GB)
    k_b_total = k_bs.sum(axis=1)
    bcol0 = np.zeros(NB + 1, dtype=np.int64)
    np.cumsum(k_b_total, out=bcol0[1:])
    pref_s = np.zeros((NB, NCHUNK + 1), dtype=np.int64)
    np.cumsum(k_bs, axis=1, out=pref_s[:, 1:])

    groups = []
    col = 0
    block_col = np.zeros((NB, NCHUNK), dtype=np.int64)
    for g in range(ngroups):
        bs = list(range(g * GB, min((g + 1) * GB, NB)))
        grp = {"col0": col, "blocks": [], "k_gs": [], "s_col0": []}
        for s in range(NCHUNK):
            grp["s_col0"].append(col)
            k_gs = 0
            for b in bs:
                block_col[b, s] = col
                col += int(k_bs[b, s])
                k_gs += int(k_bs[b, s])
            grp["k_gs"].append(k_gs)
        grp["ncols"] = col - grp["col0"]
        for b in bs:
            cols = []
            for s in range(NCHUNK):
                cols.extend(
                    range(
                        int(block_col[b, s]),
                        int(block_col[b, s]) + int(k_bs[b, s]),
                    )
                )
            grp["blocks"].append(
                {"b": b, "cols": cols, "bcol0": int(bcol0[b])}
            )
        groups.append(grp)
    G_cols = col

    layout = {"G_cols": G_cols, "groups": groups}
    cfg = dict(
        NPAD=NPAD,
        NLOCP=NLOCP,
        NLOC=NLOC,
        NB=NB,
        BN=BN,
        F=F,
        H=H,
        C=C,
        CP=CP,
        NCORES=NCORES,
        GB=GB,
        OHSLAB=OHSLAB,
    )

    iota = np.broadcast_to(
        np.tile(np.arange(BN, dtype=np.float32), OHSLAB)[None, :],
        (128, OHSLAB * BN),
    ).astype(NPBF16)
    ident = np.eye(128, dtype=NPBF16)
    b1rep = np.broadcast_to(
        np.asarray(b1, dtype=np.float32)[None, :], (128, H)
    ).copy()
    b2rep = np.broadcast_to(
        np.asarray(b2, dtype=np.float32)[None, :], (128, C)
    ).copy()
    W2p = np.zeros((H, CP), dtype=np.float32)
    W2p[:, :C] = np.asarray(W2, dtype=np.float32)

    run_start = np.zeros(NCORES * NB * NCHUNK + 1, dtype=np.int64)
    np.cumsum(counts.reshape(-1), out=run_start[1:])
    total = len(key_o)
    j_in_run = np.arange(total) - run_start[key_o]

    s_col0_arr = np.zeros((ngroups, NCHUNK), dtype=np.int64)
    for g in range(ngroups):
        for s in range(NCHUNK):
            s_col0_arr[g, s] = groups[g]["s_col0"][s]

    in_maps = []
    for cidx in range(NCORES):
        lo = run_start[cidx * NB * NCHUNK]
        hi = run_start[(cidx + 1) * NB * NCHUNK]
        sl = slice(lo, hi)
        k_loc = key_o[sl] - cidx * NB * NCHUNK
        b_loc = k_loc // NCHUNK
        s_loc = k_loc % NCHUNK
        j_loc = j_in_run[sl]
        col_abs = block_col[b_loc, s_loc] + j_loc // 128
        p_loc = j_loc % 128

        # dstloc grid in BLOCK-MAJOR columns; pad slots = -1 (never match)
        dstlocg = np.full((128, G_cols), -1.0, dtype=np.float32)
        bm_col = bcol0[b_loc] + pref_s[b_loc, s_loc] + j_loc // 128
        dstlocg[p_loc, bm_col] = within_o[sl]

        g_loc = b_loc // GB
        pos_gs = (col_abs - s_col0_arr[g_loc, s_loc]) * 128 + p_loc
        idxval = (src_o[sl] - s_loc * CH).astype(np.int16)
        idxg = np.zeros((128, 8 * G_cols), dtype=np.int16)
        rowi = (pos_gs % 16).astype(np.int64)
        coli = 8 * s_col0_arr[g_loc, s_loc] + pos_gs // 16
        idxg[rowi, coli] = idxval
        idxg16 = idxg[:16]
        for kk in range(1, 8):
            idxg[16 * kk : 16 * (kk + 1)] = idxg16

        dinvb = np.zeros((128, NB), dtype=np.float32)
        nodes = np.arange(NLOC)
        dinvb[nodes % BN, nodes // BN] = dinv[
            cidx * NLOC : (cidx + 1) * NLOC
        ]

        in_maps.append(
            {
                "x_pad": x_pad,
                "x_loc": np.ascontiguousarray(xv[cidx]),
                "idxg": idxg,
                "dstlocg": dstlocg.astype(NPBF16),
                "dinvb": dinvb,
                "W1": np.asarray(W1, dtype=np.float32).astype(NPBF16),
                "b1rep": b1rep,
                "W2": W2p.astype(NPBF16),
                "b2rep": b2rep,
                "iota": iota,
                "ident": ident,
            }
        )

    return cfg, layout, in_maps


def postprocess(cfg, results):
    NLOC = cfg["NLOC"]
    outs = [r["out"][:NLOC] for r in results]
    return np.concatenate(outs, axis=0)


# ====================== harness entrypoint ======================

_CACHE = {}
LAST_EXEC_NS = None


def kernel(**inputs):
    """Full-input GCN2 forward on 8 TRN2 NeuronCores.

    Shards nodes across the 8 cores (edges partitioned by destination),
    runs the Bass kernel via run_bass_kernel_spmd, gathers the output.
    """
    global LAST_EXEC_NS
    import os

    from concourse.bass_utils import run_bass_kernel_spmd

    x = np.asarray(inputs["x"], dtype=np.float32)
    edge_index = np.asarray(inputs["edge_index"])
    W1 = np.asarray(inputs["W1"], dtype=np.float32)
    b1 = np.asarray(inputs["b1"], dtype=np.float32)
    W2 = np.asarray(inputs["W2"], dtype=np.float32)
    b2 = np.asarray(inputs["b2"], dtype=np.float32)

    NCORES = 8
    cfg, layout, in_maps = prep(
        x, edge_index, W1, b1, W2, b2, NCORES=NCORES, GB=8
    )
    key = (
        x.shape,
        edge_index.shape,
        layout["G_cols"],
        tuple(tuple(g["k_gs"]) for g in layout["groups"]),
    )
    nc = _CACHE.get(key)
    if nc is None:
        nc = build_gcn_nc(cfg, layout)
        _CACHE[key] = nc

    trace = os.environ.get("GCN_TRACE", "0") == "1"
    res = run_bass_kernel_spmd(
        nc, in_maps, core_ids=list(range(NCORES)), trace=trace
    )
    LAST_EXEC_NS = res.exec_time_ns
    out = postprocess(cfg, res.results)
    return out.astype(np.float32)


# revision 30
# speedup vs baseline: 1.3825x; 1.3825x over previous
"""GCN 2-layer Bass kernel for TRN2, sharded over NCORES cores.

Sharding: nodes split evenly across cores; edges partitioned by destination
node; weights replicated; layer-2 source features exchanged via AllGather.

Math (per reference):
    h   = relu(Ahat @ (x @ W1) + b1)    = relu((Ahat @ x) @ W1 + b1)
    out = Ahat @ (h @ W2) + b2
where Ahat = D^-1/2 (A+I) D^-1/2 on the self-loop-augmented graph.

Factorization used on device: with x' = dinv*x (host-prescaled),
    Ahat x = dinv_dst * ((A+I) x')
so gathers read pre-scaled rows, selection matrices are pure 0/1
(is_equal, padding = -1 never matches), and the dst-side dinv is applied
where nodes sit on PSUM partitions.

Edge layout: edges are bucketed by (dst block, source chunk); within each
(group, chunk) region the per-(block,chunk) runs are packed back to back
with only max-over-cores padding (no per-block 128-alignment), so gather
columns may span adjacent blocks. A range's first column may be shared
with the previous block: its dst values are offset by BN and compared
against the 128..255 iota columns, so both blocks can matmul the shared
column safely. All tables, weights, selection matrices and gathered rows
are fp16 (integers <= 2048 exact, ~2^-11 rounding); gather-table rows are
128 fp16 elements (256B, the SWDGE element minimum) with the payload in
the first 64. PSUM accumulation stays fp32.

Performance notes: the per-edge SWDGE gathers (256B/edge/layer) dominate;
dynamic_dma_scratch_size=2**15 doubles the per-queue descriptor-ring
depth so desc-gen runs ahead of the SDMA drain, GBUFS=3 double-buffers
gathered groups, x_loc/t2 blocks stay SBUF-resident (self-loop matmuls
read SBUF, no per-block DMAs), and t2/out are staged and stored in
NSLAB block-slabs so HBM writes spread through the compute.

Device algorithm per core (owns NLOC nodes, NB blocks of BN=128 dst nodes):
  phase A: per group of GB blocks: dma_gather x' rows for the group's edges
      (one gather per source chunk, spread over 4 SWDGE queues; index/dst
      tables stream in per group). Per block: identity matmul for
      self-loops + per 128-edge column a PE matmul with the selection
      matrix (built 8 columns per DVE is_equal), accumulating q.T [64, BN]
      in PSUM. Inline node-major tail: z = (q.T).T @ W1;
      h = relu(dinv*z + b1); hT = h.T (PE); t2 = hT.T @ W2;
      t2 slab row-block = dinv*t2 (layer-2 source-side prescale).
  phase B: t2loc slab stores (as their blocks finish) + one AllGather
      t2loc -> t2full [NPAD, 128] fp16.
  phase C: same structure over t2full; self-loops read the SBUF-resident
      t2 slabs; out_block = dinv*(transpose) + b2, stored per slab.
"""

import sys

sys.path.insert(0, "/opt/trn_rl_repo")

import numpy as np

import concourse.bass as bass
import concourse.mybir as mybir
import concourse.tile as tile
from concourse import bacc

F32 = mybir.dt.float32
F32R = mybir.dt.float32r
F16 = mybir.dt.float16
I16 = mybir.dt.int16
AF = mybir.ActivationFunctionType
ALU = mybir.AluOpType

NCHUNK = 4  # source buckets per layer (int16 index range / slab count)
NQ = 4  # SWDGE queues
FW = 128  # gather-table row width in fp16 elements (256B = SWDGE minimum)


def build_gcn_nc(cfg, layouts):
    NPAD, NLOCP, NB, BN = cfg["NPAD"], cfg["NLOCP"], cfg["NB"], cfg["BN"]
    F, H, C, NCORES = cfg["F"], cfg["H"], cfg["C"], cfg["NCORES"]
    GB, OHSLAB = cfg["GB"], cfg["OHSLAB"]
    CH = NPAD // NCHUNK
    slabs = cfg["slabs"]  # list of (b0, nb)
    NSLAB = len(slabs)

    nc = bacc.Bacc(
        "TRN2",
        target_bir_lowering=False,
        debug=False,
        num_devices=NCORES,
        num_swdge_queues=NQ,
        dynamic_dma_scratch_size=2**16,
    )

    # ---------------- I/O ----------------
    G1 = layouts[0]["G_cols"]
    x_d = nc.dram_tensor("x_pad", [NPAD, FW], F16, kind="ExternalInput")
    xloc_d = nc.dram_tensor("x_loc", [NLOCP, FW], F16, kind="ExternalInput")
    idxg1_d = nc.dram_tensor("idxg1", [128, 8 * G1], I16, kind="ExternalInput")
    dst1_d = nc.dram_tensor("dst1", [128, G1], F16, kind="ExternalInput")
    dinvb_d = nc.dram_tensor("dinvb", [128, NB], F32, kind="ExternalInput")
    w1_d = nc.dram_tensor("W1", [F, H], F16, kind="ExternalInput")
    b1rep_d = nc.dram_tensor("b1rep", [128, H], F32, kind="ExternalInput")
    w2_d = nc.dram_tensor("W2", [H, C], F16, kind="ExternalInput")
    b2rep_d = nc.dram_tensor("b2rep", [128, C], F32, kind="ExternalInput")
    iota_d = nc.dram_tensor(
        "iota", [128, (OHSLAB + 1) * BN], F16, kind="ExternalInput"
    )
    ident_d = nc.dram_tensor("ident", [128, 128], F16, kind="ExternalInput")
    out_d = nc.dram_tensor("out", [NLOCP, C], F32, kind="ExternalOutput")

    qctr = [0]

    def next_q():
        q = qctr[0] % NQ
        qctr[0] += 1
        return q

    with tile.TileContext(nc) as tc:
        with (
            tc.tile_pool(name="const", bufs=1) as cstp,
            tc.tile_pool(name="dram", bufs=1, space="DRAM") as dram_pool,
            tc.tile_pool(name="gat", bufs=cfg.get("GBUFS", 2)) as gpool,
            tc.tile_pool(name="idx", bufs=2) as idxpool,
            tc.tile_pool(name="dst", bufs=2) as dstpool,
            tc.tile_pool(name="ohb", bufs=cfg.get("OHBBUFS", 3)) as ohbpool,
            tc.tile_pool(name="cp", bufs=4) as cpool,
            tc.tile_pool(name="ps_pT", bufs=2, space="PSUM") as ps_pT,
            tc.tile_pool(name="ps_h", bufs=2, space="PSUM") as ps_h,
            tc.tile_pool(name="ps_t2", bufs=2, space="PSUM") as ps_t2,
            tc.tile_pool(name="ps_tr", bufs=2, space="PSUM") as ps_tr,
        ):
            dinvb_s = cstp.tile([128, NB], F32, name="dinvb_s")
            w1_s = cstp.tile([F, H], F16, name="w1_s")
            b1rep_s = cstp.tile([128, H], F32, name="b1rep_s")
            w2_s = cstp.tile([H, C], F16, name="w2_s")
            b2rep_s = cstp.tile([128, C], F32, name="b2rep_s")
            iota_s = cstp.tile([128, (OHSLAB + 1) * BN], F16, name="iota_s")
            ident_s = cstp.tile([128, 128], F16, name="ident_s")
            outst_k = [
                cstp.tile([128, nb_ * C], F32, name=f"outst{k}")
                for k, (b0_, nb_) in enumerate(slabs)
            ]

            nc.sync.dma_start(out=dinvb_s[:], in_=dinvb_d[:])
            nc.sync.dma_start(out=w1_s[:], in_=w1_d[:])
            nc.sync.dma_start(out=b1rep_s[:], in_=b1rep_d[:])
            nc.sync.dma_start(out=w2_s[:], in_=w2_d[:])
            nc.sync.dma_start(out=b2rep_s[:], in_=b2rep_d[:])
            nc.sync.dma_start(out=iota_s[:], in_=iota_d[:])
            nc.sync.dma_start(out=ident_s[:], in_=ident_d[:])

            xloc_s = cstp.tile([128, NB * FW], F16, name="xloc_s")
            nc.scalar.dma_start(
                out=xloc_s[:].rearrange("p (b c) -> p b c", b=NB),
                in_=xloc_d.rearrange("(b p) c -> p b c", p=128),
            )

            t2loc = dram_pool.tile([NLOCP, FW], F16, name="t2loc")
            t2full = dram_pool.tile(
                [NPAD, FW], F16, name="t2full", addr_space="Shared"
            )

            def slab_of(b):
                for k, (b0_, nb_) in enumerate(slabs):
                    if b0_ <= b < b0_ + nb_:
                        return k, b - b0_
                raise AssertionError(b)

            def emit_group_gathers(
                grp, idxg_d_, dst_d_, src_of, prepare=False, sem=None
            ):
                """Issue the idx/dst loads + chunk gathers for one group.

                With prepare=True the gathers only write SWDGE descriptors
                (data deps defer to a later trigger_dma on each queue).
                Returns (xg, dst_t, used_queues)."""
                g0 = grp["col0"]
                gcols = grp["ncols"]
                xg = gpool.tile([128, gcols * FW], F16, tag="xg")
                idxg_t = idxpool.tile([128, 8 * gcols], I16, tag="ix")
                dst_t = dstpool.tile([128, gcols], F16, tag="dl")
                nc.sync.dma_start(
                    out=idxg_t[:],
                    in_=idxg_d_[:, 8 * g0 : 8 * (g0 + gcols)],
                )
                nc.sync.dma_start(
                    out=dst_t[:], in_=dst_d_[:, g0 : g0 + gcols]
                )
                used_q = []
                for s in range(NCHUNK):
                    k_gs = grp["k_gs"][s]
                    if k_gs == 0:
                        continue
                    c0 = grp["s_col0"][s]
                    n = 128 * k_gs
                    q = next_q()
                    used_q.append(q)
                    psem = (
                        nc.alloc_semaphore(f"{sem}q{q}") if prepare else None
                    )
                    nc.gpsimd.dma_gather(
                        out_ap=xg[
                            :, (c0 - g0) * FW : (c0 - g0 + k_gs) * FW
                        ].rearrange("p (c f) -> p c f", f=FW),
                        in_ap=src_of(s),
                        idxs_ap=idxg_t[
                            :, 8 * (c0 - g0) : 8 * (c0 - g0 + k_gs)
                        ],
                        num_idxs=n,
                        num_idxs_reg=n,
                        elem_size=FW,
                        single_packet=(n <= 1024),
                        queue_num=q,
                        prepare_only=prepare,
                        sem=psem,
                    )
                return xg, dst_t, used_q

            def layer_pass(
                lay, layout, idxg_d_, dst_d_, src_of, srcloc_of,
                t2slab_tiles=None, after_block=None, pre_emitted=None,
            ):
                for gi, grp in enumerate(layout["groups"]):
                    g0 = grp["col0"]
                    if pre_emitted is not None and gi in pre_emitted:
                        xg, dst_t = pre_emitted[gi]
                    else:
                        xg, dst_t, _ = emit_group_gathers(
                            grp, idxg_d_, dst_d_, src_of
                        )
                    for blk in grp["blocks"]:
                        b = blk["b"]
                        bg = blk["bg"]
                        ranges = blk["ranges"]
                        ncols = sum(hi - lo for lo, hi, _ in ranges)
                        # self-loops: local rows (SBUF-resident) with
                        # identity weights
                        acc = ps_pT.tile([F, BN], F32, tag="pT")
                        nc.tensor.matmul(
                            acc[:],
                            lhsT=srcloc_of(b),
                            rhs=ident_s[:, :BN],
                            start=True,
                            stop=(ncols == 0),
                        )
                        # selection matrices, up to OHSLAB cols per DVE op.
                        # A range's first column may be shared with the
                        # previous block: its dst values are offset by BN and
                        # the iota view starts at the 128..255 column.
                        done = 0
                        for lo, hi, sh in ranges:
                            for sl0 in range(lo, hi, OHSLAB):
                                slw = min(OHSLAB, hi - sl0)
                                off0 = 0 if (sh and sl0 == lo) else BN
                                ohb = ohbpool.tile(
                                    [128, OHSLAB * BN], F16, tag="ohb"
                                )
                                nc.vector.tensor_tensor(
                                    out=ohb[:, : slw * BN].rearrange(
                                        "p (k n) -> p k n", n=BN
                                    ),
                                    in0=iota_s[
                                        :, off0 : off0 + slw * BN
                                    ].rearrange("p (k n) -> p k n", n=BN),
                                    in1=dst_t[
                                        :, sl0 - g0 : sl0 - g0 + slw, None
                                    ].to_broadcast([128, slw, BN]),
                                    op=ALU.is_equal,
                                )
                                for i in range(slw):
                                    col = sl0 + i
                                    done += 1
                                    last = done == ncols
                                    nc.tensor.matmul(
                                        acc[:],
                                        lhsT=xg[
                                            :,
                                            (col - g0) * FW : (col - g0)
                                            * FW
                                            + F,
                                        ],
                                        rhs=ohb[:, i * BN : (i + 1) * BN],
                                        start=False,
                                        stop=last,
                                    )
                        if lay == 0:
                            # inline node-major dense tail for this block
                            qsb = cpool.tile([F, BN], F16, tag="qsb")
                            nc.scalar.copy(out=qsb[:], in_=acc[:])
                            z_ps = ps_h.tile([BN, H], F32, tag="z")
                            nc.tensor.matmul(
                                z_ps[:],
                                lhsT=qsb[:],
                                rhs=w1_s[:],
                                start=True,
                                stop=True,
                            )
                            h_sb = cpool.tile([BN, H], F32, tag="h")
                            nc.scalar.activation(
                                out=h_sb[:],
                                in_=z_ps[:],
                                func=AF.Copy,
                                bias=0.0,
                                scale=dinvb_s[:, b : b + 1],
                            )
                            h2_sb = cpool.tile([BN, H], F32, tag="h2")
                            nc.vector.tensor_tensor(
                                out=h2_sb[:],
                                in0=h_sb[:],
                                in1=b1rep_s[:],
                                op=ALU.add,
                            )
                            hr_sb = cpool.tile([BN, H], F16, tag="hr")
                            nc.scalar.activation(
                                out=hr_sb[:],
                                in_=h2_sb[:],
                                func=AF.Relu,
                                bias=0.0,
                                scale=1.0,
                            )
                            hT_ps = ps_tr.tile([H, BN], F16, tag="tr")
                            nc.tensor.matmul(
                                hT_ps[:],
                                lhsT=hr_sb[:],
                                rhs=ident_s[:, :BN],
                                is_transpose=True,
                                start=True,
                                stop=True,
                            )
                            hT_sb = cpool.tile([H, BN], F16, tag="hTs")
                            nc.scalar.copy(out=hT_sb[:], in_=hT_ps[:])
                            t2_ps = ps_t2.tile([BN, C], F32, tag="t2")
                            nc.tensor.matmul(
                                t2_ps[:],
                                lhsT=hT_sb[:],
                                rhs=w2_s[:],
                                start=True,
                                stop=True,
                            )
                            # t2 row pre-scaled by dinv (layer-2 src side)
                            k, boff = slab_of(b)
                            nc.scalar.activation(
                                out=t2slab_tiles[k][
                                    :, boff * FW : boff * FW + C
                                ],
                                in_=t2_ps[:],
                                func=AF.Copy,
                                bias=0.0,
                                scale=dinvb_s[:, b : b + 1],
                            )
                        else:
                            o2 = cpool.tile([F, BN], F16, tag="o2")
                            nc.scalar.copy(out=o2[:], in_=acc[:])
                            tr = ps_tr.tile([BN, F], F16, tag="tr")
                            nc.tensor.matmul(
                                tr[:],
                                lhsT=o2[:],
                                rhs=ident_s[:F, :F],
                                is_transpose=True,
                                start=True,
                                stop=True,
                            )
                            y = cpool.tile([BN, C], F32, tag="y")
                            nc.scalar.activation(
                                out=y[:],
                                in_=tr[:, :C],
                                func=AF.Copy,
                                bias=0.0,
                                scale=dinvb_s[:, b : b + 1],
                            )
                            k, boff = slab_of(b)
                            nc.vector.tensor_tensor(
                                out=outst_k[k][
                                    :, boff * C : (boff + 1) * C
                                ],
                                in0=y[:],
                                in1=b2rep_s[:],
                                op=ALU.add,
                            )
                        if after_block is not None:
                            after_block(b)

            # ---------------- phase A (+ overlapped B) ----------------
            t2slab_tiles = []
            slab_done = [False] * NSLAB

            def maybe_fire_slabs(done_b):
                """Store any slab whose blocks are all done (spreads the
                t2loc HBM writes through phase A)."""
                for k, (b0_, nb_) in enumerate(slabs):
                    if slab_done[k] or done_b < b0_ + nb_ - 1:
                        continue
                    slab_done[k] = True
                    st = t2slab_tiles[k]
                    nc.sync.dma_start(
                        out=t2loc[b0_ * BN : (b0_ + nb_) * BN, :].rearrange(
                            "(b p) c -> p b c", p=128
                        ),
                        in_=st[:].rearrange("p (b c) -> p b c", b=nb_),
                    )

            phases = cfg.get("PHASES", "A,B,C").split(",")
            if "A" in phases:
                for k, (b0_, nb_) in enumerate(slabs):
                    st = cstp.tile([128, nb_ * FW], F16, name=f"t2st{k}")
                    t2slab_tiles.append(st)

                layer_pass(
                    0,
                    layouts[0],
                    idxg1_d,
                    dst1_d,
                    lambda s: x_d[s * CH : (s + 1) * CH, :],
                    lambda b: xloc_s[:, b * FW : b * FW + F],
                    t2slab_tiles,
                    after_block=maybe_fire_slabs,
                )
                maybe_fire_slabs(NB - 1)

            # ------- L2 gather prefetch: write descriptors during B -------
            PREFETCH = cfg.get("PREFETCH", 0)
            pre_emitted = {}
            pre_queues = set()
            if "C" in phases and PREFETCH > 0:
                for gi in range(min(PREFETCH, len(layouts[0]["groups"]))):
                    grp = layouts[0]["groups"][gi]
                    xg, dst_t, used_q = emit_group_gathers(
                        grp,
                        idxg1_d,
                        dst1_d,
                        lambda s: t2full[s * CH : (s + 1) * CH, :],
                        prepare=True,
                        sem=f"l2pre{gi}",
                    )
                    pre_emitted[gi] = (xg, dst_t)
                    pre_queues.update(used_q)

            # ---------------- phase B: AllGather ----------------
            if "B" in phases:
                if NCORES > 1:
                    nc.gpsimd.collective_compute(
                        "AllGather",
                        ALU.bypass,
                        replica_groups=[list(range(NCORES))],
                        ins=[t2loc[:, :]],
                        outs=[t2full[:, :]],
                    )
                else:
                    nc.sync.dma_start(out=t2full[:, :], in_=t2loc[:, :])
                for q in sorted(pre_queues):
                    nc.gpsimd.trigger_dma(count=None, queue_num=q)

            # ---------------- phase C ----------------
            if "C" in phases:
                out_done = [False] * NSLAB

                def maybe_store_out(done_b):
                    for k, (b0_, nb_) in enumerate(slabs):
                        if out_done[k] or done_b < b0_ + nb_ - 1:
                            continue
                        out_done[k] = True
                        nc.sync.dma_start(
                            out=out_d[
                                b0_ * BN : (b0_ + nb_) * BN, :
                            ].rearrange("(b p) c -> p b c", p=128),
                            in_=outst_k[k][:].rearrange(
                                "p (b c) -> p b c", b=nb_
                            ),
                        )

                def srcloc2(b):
                    k, boff = slab_of(b)
                    return t2slab_tiles[k][:, boff * FW : boff * FW + F]

                layer_pass(
                    1,
                    layouts[0],
                    idxg1_d,
                    dst1_d,
                    lambda s: t2full[s * CH : (s + 1) * CH, :],
                    srcloc2,
                    after_block=maybe_store_out,
                    pre_emitted=pre_emitted,
                )
                maybe_store_out(NB - 1)

    nc.compile()
    return nc


# ====================== host-side preprocessing ======================


def _build_layout(
    src_bucket, src_idx, blk, within, core, NCORES, NB, GB, order_key_extra=None
):
    """Bucket edges by (core, block, bucket), pack densely per (group,
    bucket) region with max-over-cores run lengths.

    src_bucket: per-edge bucket id in [0, NCHUNK)
    src_idx: per-edge row index within its bucket's table
    Returns (G_cols, groups, per-core (idxg, dstlocg)).
    """
    E = len(src_bucket)
    key = ((core * NB + blk) * NCHUNK + src_bucket).astype(np.int64)
    order = np.argsort(key, kind="stable")
    key_o = key[order]
    idx_o = src_idx[order]
    within_o = within[order]
    blk_o = blk[order]

    counts = np.bincount(key_o, minlength=NCORES * NB * NCHUNK).reshape(
        NCORES, NB, NCHUNK
    )
    cnt_max = counts.max(axis=0)

    ngroups = -(-NB // GB)
    groups = []
    col = 0
    run_off = np.zeros((NB, NCHUNK), dtype=np.int64)
    reg_col0 = np.zeros((ngroups, NCHUNK), dtype=np.int64)
    for g in range(ngroups):
        bs = list(range(g * GB, min((g + 1) * GB, NB)))
        grp = {"col0": col, "blocks": [], "k_gs": [], "s_col0": [], "g": g}
        rng_by_block = {b: [] for b in bs}
        for s in range(NCHUNK):
            grp["s_col0"].append(col)
            reg_col0[g, s] = col
            off = 0
            for b in bs:
                c = int(cnt_max[b, s])
                run_off[b, s] = off
                if c > 0:
                    lo = col + off // 128
                    hi = col + -(-(off + c) // 128)
                    rng_by_block[b].append((lo, hi, off % 128 != 0))
                off += c
            k_gs = -(-off // 128)
            col += k_gs
            grp["k_gs"].append(k_gs)
        grp["ncols"] = col - grp["col0"]
        for b in bs:
            grp["blocks"].append(
                {"b": b, "bg": b - g * GB, "ranges": rng_by_block[b]}
            )
        groups.append(grp)
    G_cols = col

    run_start = np.zeros(NCORES * NB * NCHUNK + 1, dtype=np.int64)
    np.cumsum(counts.reshape(-1), out=run_start[1:])
    total = len(key_o)
    j_in_run = np.arange(total) - run_start[key_o]

    per_core = []
    BN = 128
    for cidx in range(NCORES):
        lo = run_start[cidx * NB * NCHUNK]
        hi = run_start[(cidx + 1) * NB * NCHUNK]
        sl = slice(lo, hi)
        k_loc = key_o[sl] - cidx * NB * NCHUNK
        b_loc = k_loc // NCHUNK
        s_loc = k_loc % NCHUNK
        j_loc = j_in_run[sl]
        g_loc = b_loc // GB
        pos = run_off[b_loc, s_loc] + j_loc
        col_abs = reg_col0[g_loc, s_loc] + pos // 128
        p_loc = pos % 128

        # dst value = within + BN if the edge sits in a column whose first
        # slot belongs to the previous block's run (shared boundary column)
        shared = (run_off[b_loc, s_loc] % 128 != 0) & (
            pos // 128 == run_off[b_loc, s_loc] // 128
        )
        dstlocg = np.full((128, G_cols), -1.0, dtype=np.float32)
        dstlocg[p_loc, col_abs] = within_o[sl] + BN * shared

        idxval = idx_o[sl].astype(np.int16)
        idxg = np.zeros((128, 8 * G_cols), dtype=np.int16)
        rowi = (pos % 16).astype(np.int64)
        coli = 8 * reg_col0[g_loc, s_loc] + pos // 16
        idxg[rowi, coli] = idxval
        idxg16 = idxg[:16]
        for kk in range(1, 8):
            idxg[16 * kk : 16 * (kk + 1)] = idxg16
        per_core.append((idxg, dstlocg.astype(np.float16)))

    layout = {"G_cols": G_cols, "groups": groups}
    return layout, per_core


def prep(
    x, edge_index, W1, b1, W2, b2, NCORES=8, BN=128, GB=5, OHSLAB=8, NSLAB=4
):
    """Partition/pad inputs. Returns (cfg, layouts, in_maps)."""
    N, F = x.shape
    H = W1.shape[1]
    C = W2.shape[1]
    assert N % NCORES == 0
    NLOC = N // NCORES
    NB = -(-NLOC // BN)
    NLOCP = NB * BN
    NPAD = NCORES * NLOCP
    assert NPAD % NCHUNK == 0
    CH = NPAD // NCHUNK
    assert CH <= 32768, "chunk exceeds int16 index range"

    src = np.asarray(edge_index[0], dtype=np.int64)
    dst = np.asarray(edge_index[1], dtype=np.int64)

    deg = np.bincount(dst, minlength=N).astype(np.float64) + 1.0
    dinv = (1.0 / np.sqrt(deg)).astype(np.float32)

    # pre-scale x by dinv (source-side factor of Ahat)
    xs = np.asarray(x, dtype=np.float32) * dinv[:, None]
    x_pad = np.zeros((NPAD, FW), dtype=np.float16)
    xv = x_pad.reshape(NCORES, NLOCP, FW)
    xv[:, :NLOC, :F] = xs.reshape(NCORES, NLOC, F).astype(np.float16)
    src_pad = src + (NLOCP - NLOC) * (src // NLOC)

    core = dst // NLOC
    dstloc = dst - core * NLOC
    blk = dstloc // BN
    within = (dstloc % BN).astype(np.float32)

    # slabs of blocks (staged t2 stores during phase A)
    bps = -(-NB // NSLAB)
    slabs = []
    b0 = 0
    while b0 < NB:
        nb_ = min(bps, NB - b0)
        slabs.append((b0, nb_))
        b0 += nb_

    # shared layout for both layers: bucket = src table quarter
    lay1, pc1 = _build_layout(
        src_pad // CH, src_pad % CH, blk, within, core, NCORES, NB, GB
    )
    layouts = [lay1]
    cfg = dict(
        NPAD=NPAD,
        NLOCP=NLOCP,
        NLOC=NLOC,
        NB=NB,
        BN=BN,
        F=F,
        H=H,
        C=C,
        NCORES=NCORES,
        GB=GB,
        OHSLAB=OHSLAB,
        slabs=slabs,
    )

    # iota: first BN columns are 128..255 (leading shared column), the
    # remaining OHSLAB copies are 0..127
    iota = np.zeros((128, (OHSLAB + 1) * BN), dtype=np.float16)
    iota[:, :BN] = (BN + np.arange(BN, dtype=np.float16))[None, :]
    iota[:, BN:] = np.tile(np.arange(BN, dtype=np.float16), OHSLAB)[None, :]
    ident = np.eye(128, dtype=np.float16)
    b1rep = np.broadcast_to(
        np.asarray(b1, dtype=np.float32)[None, :], (128, H)
    ).copy()
    b2rep = np.broadcast_to(
        np.asarray(b2, dtype=np.float32)[None, :], (128, C)
    ).copy()

    in_maps = []
    for cidx in range(NCORES):
        dinvb = np.zeros((128, NB), dtype=np.float32)
        nodes = np.arange(NLOC)
        dinvb[nodes % BN, nodes // BN] = dinv[
            cidx * NLOC : (cidx + 1) * NLOC
        ]
        in_maps.append(
            {
                "x_pad": x_pad,
                "x_loc": np.ascontiguousarray(xv[cidx]),
                "idxg1": pc1[cidx][0],
                "dst1": pc1[cidx][1],
                "dinvb": dinvb,
                "W1": np.asarray(W1, dtype=np.float16),
                "b1rep": b1rep,
                "W2": np.asarray(W2, dtype=np.float16),
                "b2rep": b2rep,
                "iota": iota,
                "ident": ident,
            }
        )

    return cfg, layouts, in_maps


def postprocess(cfg, results):
    NLOC = cfg["NLOC"]
    outs = [r["out"][:NLOC] for r in results]
    return np.concatenate(outs, axis=0)


# ====================== harness entrypoint ======================

_CACHE = {}
LAST_EXEC_NS = None


def kernel(**inputs):
    """Full-input GCN2 forward on 8 TRN2 NeuronCores.

    Shards nodes across the 8 cores (edges partitioned by destination),
    runs the Bass kernel via run_bass_kernel_spmd, gathers the output.
    """
    global LAST_EXEC_NS
    import os

    from concourse.bass_utils import run_bass_kernel_spmd

    x = np.asarray(inputs["x"], dtype=np.float32)
    edge_index = np.asarray(inputs["edge_index"])
    W1 = np.asarray(inputs["W1"], dtype=np.float32)
    b1 = np.asarray(inputs["b1"], dtype=np.float32)
    W2 = np.asarray(inputs["W2"], dtype=np.float32)
    b2 = np.asarray(inputs["b2"], dtype=np.float32)

    NCORES = 8
    cfg, layouts, in_maps = prep(
        x, edge_index, W1, b1, W2, b2, NCORES=NCORES
    )
    key = (
        x.shape,
        edge_index.shape,
        layouts[0]["G_cols"],
        tuple(tuple(g["k_gs"]) for g in layouts[0]["groups"]),
    )
    nc = _CACHE.get(key)
    if nc is None:
        nc = build_gcn_nc(cfg, layouts)
        _CACHE[key] = nc

    trace = os.environ.get("GCN_TRACE", "0") == "1"
    res = run_bass_kernel_spmd(
        nc, in_maps, core_ids=list(range(NCORES)), trace=trace
    )
    LAST_EXEC_NS = res.exec_time_ns
    out = postprocess(cfg, res.results)
    return out.astype(np.float32)


# revision 34
# speedup vs baseline: 1.5270x; 1.1045x over previous
"""GCN 2-layer Bass kernel for TRN2, sharded over NCORES cores.

Sharding: nodes split evenly across cores; edges partitioned by destination
node; weights replicated; layer-2 source features exchanged via AllGather.

Math (per reference):
    h   = relu(Ahat @ (x @ W1) + b1)    = relu((Ahat @ x) @ W1 + b1)
    out = Ahat @ (h @ W2) + b2
where Ahat = D^-1/2 (A+I) D^-1/2 on the self-loop-augmented graph.

Factorization used on device: with x' = dinv*x (host-prescaled),
    Ahat x = dinv_dst * ((A+I) x')
so gathers read pre-scaled rows, selection matrices are pure 0/1
(is_equal, padding = -1 never matches), and the dst-side dinv is applied
where nodes sit on PSUM partitions.

Edge layout: edges are bucketed by (dst block, source chunk); within each
(group, chunk) region the per-(block,chunk) runs are packed back to back
with only max-over-cores padding (no per-block 128-alignment), so gather
columns may span adjacent blocks. A range's first column may be shared
with the previous block: its dst values are offset by BN and compared
against the 128..255 iota columns, so both blocks can matmul the shared
column safely. All tables, weights, selection matrices and gathered rows
are fp16 (integers <= 2048 exact, ~2^-11 rounding); gather-table rows are
128 fp16 elements (256B, the SWDGE element minimum) with the payload in
the first 64. PSUM accumulation stays fp32.

Performance notes: the per-edge SWDGE gathers (256B/edge/layer) dominate;
dynamic_dma_scratch_size=2**15 doubles the per-queue descriptor-ring
depth so desc-gen runs ahead of the SDMA drain, GBUFS=3 double-buffers
gathered groups, x_loc/t2 blocks stay SBUF-resident (self-loop matmuls
read SBUF, no per-block DMAs), and t2/out are staged and stored in
NSLAB block-slabs so HBM writes spread through the compute.

Device algorithm per core (owns NLOC nodes, NB blocks of BN=128 dst nodes):
  phase A: per group of GB blocks: dma_gather x' rows for the group's edges
      (one gather per source chunk, spread over 4 SWDGE queues; index/dst
      tables stream in per group). Per block: identity matmul for
      self-loops + per 128-edge column a PE matmul with the selection
      matrix (built 8 columns per DVE is_equal), accumulating q.T [64, BN]
      in PSUM. Inline node-major tail: z = (q.T).T @ W1;
      h = relu(dinv*z + b1); hT = h.T (PE); t2 = hT.T @ W2;
      t2 slab row-block = dinv*t2 (layer-2 source-side prescale).
  phase B: t2loc slab stores (as their blocks finish) + one AllGather
      t2loc -> t2full [NPAD, 128] fp16.
  phase C: same structure over t2full; self-loops read the SBUF-resident
      t2 slabs; out_block = dinv*(transpose) + b2, stored per slab.
"""

import sys

sys.path.insert(0, "/opt/trn_rl_repo")

import numpy as np

import concourse.bass as bass
import concourse.mybir as mybir
import concourse.tile as tile
from concourse import bacc

F32 = mybir.dt.float32
F32R = mybir.dt.float32r
F16 = mybir.dt.float16
I16 = mybir.dt.int16
AF = mybir.ActivationFunctionType
ALU = mybir.AluOpType

NCHUNK = 4  # source buckets per layer (int16 index range / slab count)
NQ = 4  # SWDGE queues
FW = 128  # gather-table row width in fp16 elements (256B = SWDGE minimum)


def build_gcn_nc(cfg, layouts):
    NPAD, NLOCP, NB, BN = cfg["NPAD"], cfg["NLOCP"], cfg["NB"], cfg["BN"]
    F, H, C, NCORES = cfg["F"], cfg["H"], cfg["C"], cfg["NCORES"]
    GB, OHSLAB = cfg["GB"], cfg["OHSLAB"]
    CH = NPAD // NCHUNK
    slabs = cfg["slabs"]  # list of (b0, nb)
    NSLAB = len(slabs)

    nc = bacc.Bacc(
        "TRN2",
        target_bir_lowering=False,
        debug=False,
        num_devices=NCORES,
        num_swdge_queues=NQ,
        dynamic_dma_scratch_size=2**15,
    )

    # ---------------- I/O ----------------
    G1 = layouts[0]["G_cols"]
    x_d = nc.dram_tensor("x_pad", [NPAD, FW], F16, kind="ExternalInput")
    xloc_d = nc.dram_tensor("x_loc", [NLOCP, FW], F16, kind="ExternalInput")
    idxg1_d = nc.dram_tensor("idxg1", [128, 8 * G1], I16, kind="ExternalInput")
    dst1_d = nc.dram_tensor("dst1", [128, G1], F16, kind="ExternalInput")
    dinvb_d = nc.dram_tensor("dinvb", [128, NB], F32, kind="ExternalInput")
    w1_d = nc.dram_tensor("W1", [F, H], F16, kind="ExternalInput")
    b1rep_d = nc.dram_tensor("b1rep", [128, H], F32, kind="ExternalInput")
    w2_d = nc.dram_tensor("W2", [H, C], F16, kind="ExternalInput")
    b2rep_d = nc.dram_tensor("b2rep", [128, C], F32, kind="ExternalInput")
    iota_d = nc.dram_tensor(
        "iota", [128, (OHSLAB + 1) * BN], F16, kind="ExternalInput"
    )
    ident_d = nc.dram_tensor("ident", [128, 128], F16, kind="ExternalInput")
    out_d = nc.dram_tensor("out", [NLOCP, C], F32, kind="ExternalOutput")

    qctr = [0]

    def next_q():
        q = qctr[0] % NQ
        qctr[0] += 1
        return q

    with tile.TileContext(nc) as tc:
        with (
            tc.tile_pool(name="const", bufs=1) as cstp,
            tc.tile_pool(name="dram", bufs=1, space="DRAM") as dram_pool,
            tc.tile_pool(name="gat", bufs=cfg.get("GBUFS", 3)) as gpool,
            tc.tile_pool(name="idx", bufs=2) as idxpool,
            tc.tile_pool(name="dst", bufs=2) as dstpool,
            tc.tile_pool(name="ohb", bufs=cfg.get("OHBBUFS", 3)) as ohbpool,
            tc.tile_pool(name="cp", bufs=4) as cpool,
            tc.tile_pool(name="ps_pT", bufs=2, space="PSUM") as ps_pT,
            tc.tile_pool(name="ps_h", bufs=2, space="PSUM") as ps_h,
            tc.tile_pool(name="ps_t2", bufs=2, space="PSUM") as ps_t2,
            tc.tile_pool(name="ps_tr", bufs=2, space="PSUM") as ps_tr,
        ):
            dinvb_s = cstp.tile([128, NB], F32, name="dinvb_s")
            w1_s = cstp.tile([F, H], F16, name="w1_s")
            b1rep_s = cstp.tile([128, H], F32, name="b1rep_s")
            w2_s = cstp.tile([H, C], F16, name="w2_s")
            b2rep_s = cstp.tile([128, C], F32, name="b2rep_s")
            iota_s = cstp.tile([128, (OHSLAB + 1) * BN], F16, name="iota_s")
            ident_s = cstp.tile([128, 128], F16, name="ident_s")
            outst_k = [
                cstp.tile([128, nb_ * C], F32, name=f"outst{k}")
                for k, (b0_, nb_) in enumerate(slabs)
            ]

            nc.sync.dma_start(out=dinvb_s[:], in_=dinvb_d[:])
            nc.sync.dma_start(out=w1_s[:], in_=w1_d[:])
            nc.sync.dma_start(out=b1rep_s[:], in_=b1rep_d[:])
            nc.sync.dma_start(out=w2_s[:], in_=w2_d[:])
            nc.sync.dma_start(out=b2rep_s[:], in_=b2rep_d[:])
            nc.sync.dma_start(out=iota_s[:], in_=iota_d[:])
            nc.sync.dma_start(out=ident_s[:], in_=ident_d[:])

            xloc_s = cstp.tile([128, NB * FW], F16, name="xloc_s")
            nc.scalar.dma_start(
                out=xloc_s[:].rearrange("p (b c) -> p b c", b=NB),
                in_=xloc_d.rearrange("(b p) c -> p b c", p=128),
            )

            t2loc = dram_pool.tile([NLOCP, FW], F16, name="t2loc")
            t2full = dram_pool.tile(
                [NPAD, FW], F16, name="t2full", addr_space="Shared"
            )

            def slab_of(b):
                for k, (b0_, nb_) in enumerate(slabs):
                    if b0_ <= b < b0_ + nb_:
                        return k, b - b0_
                raise AssertionError(b)

            def emit_group_gathers(
                grp, idxg_d_, dst_d_, src_of, prepare=False, sem=None
            ):
                """Issue the idx/dst loads + chunk gathers for one group.

                With prepare=True the gathers only write SWDGE descriptors
                (data deps defer to a later trigger_dma on each queue).
                Returns (xg, dst_t, used_queues)."""
                g0 = grp["col0"]
                gcols = grp["ncols"]
                xg = gpool.tile([128, gcols * FW], F16, tag="xg")
                idxg_t = idxpool.tile([128, 8 * gcols], I16, tag="ix")
                dst_t = dstpool.tile([128, gcols], F16, tag="dl")
                nc.sync.dma_start(
                    out=idxg_t[:],
                    in_=idxg_d_[:, 8 * g0 : 8 * (g0 + gcols)],
                )
                nc.sync.dma_start(
                    out=dst_t[:], in_=dst_d_[:, g0 : g0 + gcols]
                )
                used_q = []
                for s in range(NCHUNK):
                    k_gs = grp["k_gs"][s]
                    if k_gs == 0:
                        continue
                    c0 = grp["s_col0"][s]
                    n = 128 * k_gs
                    q = next_q()
                    used_q.append(q)
                    psem = (
                        nc.alloc_semaphore(f"{sem}q{q}") if prepare else None
                    )
                    nc.gpsimd.dma_gather(
                        out_ap=xg[
                            :, (c0 - g0) * FW : (c0 - g0 + k_gs) * FW
                        ].rearrange("p (c f) -> p c f", f=FW),
                        in_ap=src_of(s),
                        idxs_ap=idxg_t[
                            :, 8 * (c0 - g0) : 8 * (c0 - g0 + k_gs)
                        ],
                        num_idxs=n,
                        num_idxs_reg=n,
                        elem_size=FW,
                        single_packet=(n <= 1024),
                        queue_num=q,
                        prepare_only=prepare,
                        sem=psem,
                    )
                return xg, dst_t, used_q

            def layer_pass(
                lay, layout, idxg_d_, dst_d_, src_of, srcloc_of,
                t2slab_tiles=None, after_block=None, pre_emitted=None,
            ):
                for gi, grp in enumerate(layout["groups"]):
                    g0 = grp["col0"]
                    if pre_emitted is not None and gi in pre_emitted:
                        xg, dst_t = pre_emitted[gi]
                    else:
                        xg, dst_t, _ = emit_group_gathers(
                            grp, idxg_d_, dst_d_, src_of
                        )
                    for blk in grp["blocks"]:
                        b = blk["b"]
                        bg = blk["bg"]
                        ranges = blk["ranges"]
                        ncols = sum(hi - lo for lo, hi, _ in ranges)
                        # self-loops: local rows (SBUF-resident) with
                        # identity weights
                        acc = ps_pT.tile([F, BN], F32, tag="pT")
                        nc.tensor.matmul(
                            acc[:],
                            lhsT=srcloc_of(b),
                            rhs=ident_s[:, :BN],
                            start=True,
                            stop=(ncols == 0),
                        )
                        # selection matrices, up to OHSLAB cols per DVE op.
                        # A range's first column may be shared with the
                        # previous block: its dst values are offset by BN and
                        # the iota view starts at the 128..255 column.
                        done = 0
                        for lo, hi, sh in ranges:
                            for sl0 in range(lo, hi, OHSLAB):
                                slw = min(OHSLAB, hi - sl0)
                                off0 = 0 if (sh and sl0 == lo) else BN
                                ohb = ohbpool.tile(
                                    [128, OHSLAB * BN], F16, tag="ohb"
                                )
                                nc.vector.tensor_tensor(
                                    out=ohb[:, : slw * BN].rearrange(
                                        "p (k n) -> p k n", n=BN
                                    ),
                                    in0=iota_s[
                                        :, off0 : off0 + slw * BN
                                    ].rearrange("p (k n) -> p k n", n=BN),
                                    in1=dst_t[
                                        :, sl0 - g0 : sl0 - g0 + slw, None
                                    ].to_broadcast([128, slw, BN]),
                                    op=ALU.is_equal,
                                )
                                for i in range(slw):
                                    col = sl0 + i
                                    done += 1
                                    last = done == ncols
                                    nc.tensor.matmul(
                                        acc[:],
                                        lhsT=xg[
                                            :,
                                            (col - g0) * FW : (col - g0)
                                            * FW
                                            + F,
                                        ],
                                        rhs=ohb[:, i * BN : (i + 1) * BN],
                                        start=False,
                                        stop=last,
                                    )
                        if lay == 0:
                            # inline node-major dense tail for this block
                            qsb = cpool.tile([F, BN], F16, tag="qsb")
                            nc.scalar.copy(out=qsb[:], in_=acc[:])
                            z_ps = ps_h.tile([BN, H], F32, tag="z")
                            nc.tensor.matmul(
                                z_ps[:],
                                lhsT=qsb[:],
                                rhs=w1_s[:],
                                start=True,
                                stop=True,
                            )
                            h_sb = cpool.tile([BN, H], F32, tag="h")
                            nc.scalar.activation(
                                out=h_sb[:],
                                in_=z_ps[:],
                                func=AF.Copy,
                                bias=0.0,
                                scale=dinvb_s[:, b : b + 1],
                            )
                            h2_sb = cpool.tile([BN, H], F32, tag="h2")
                            nc.vector.tensor_tensor(
                                out=h2_sb[:],
                                in0=h_sb[:],
                                in1=b1rep_s[:],
                                op=ALU.add,
                            )
                            hr_sb = cpool.tile([BN, H], F16, tag="hr")
                            nc.scalar.activation(
                                out=hr_sb[:],
                                in_=h2_sb[:],
                                func=AF.Relu,
                                bias=0.0,
                                scale=1.0,
                            )
                            hT_ps = ps_tr.tile([H, BN], F16, tag="tr")
                            nc.tensor.matmul(
                                hT_ps[:],
                                lhsT=hr_sb[:],
                                rhs=ident_s[:, :BN],
                                is_transpose=True,
                                start=True,
                                stop=True,
                            )
                            hT_sb = cpool.tile([H, BN], F16, tag="hTs")
                            nc.scalar.copy(out=hT_sb[:], in_=hT_ps[:])
                            t2_ps = ps_t2.tile([BN, C], F32, tag="t2")
                            nc.tensor.matmul(
                                t2_ps[:],
                                lhsT=hT_sb[:],
                                rhs=w2_s[:],
                                start=True,
                                stop=True,
                            )
                            # t2 row pre-scaled by dinv (layer-2 src side)
                            k, boff = slab_of(b)
                            nc.scalar.activation(
                                out=t2slab_tiles[k][
                                    :, boff * FW : boff * FW + C
                                ],
                                in_=t2_ps[:],
                                func=AF.Copy,
                                bias=0.0,
                                scale=dinvb_s[:, b : b + 1],
                            )
                        else:
                            o2 = cpool.tile([F, BN], F16, tag="o2")
                            nc.scalar.copy(out=o2[:], in_=acc[:])
                            tr = ps_tr.tile([BN, F], F16, tag="tr")
                            nc.tensor.matmul(
                                tr[:],
                                lhsT=o2[:],
                                rhs=ident_s[:F, :F],
                                is_transpose=True,
                                start=True,
                                stop=True,
                            )
                            y = cpool.tile([BN, C], F32, tag="y")
                            nc.scalar.activation(
                                out=y[:],
                                in_=tr[:, :C],
                                func=AF.Copy,
                                bias=0.0,
                                scale=dinvb_s[:, b : b + 1],
                            )
                            k, boff = slab_of(b)
                            nc.vector.tensor_tensor(
                                out=outst_k[k][
                                    :, boff * C : (boff + 1) * C
                                ],
                                in0=y[:],
                                in1=b2rep_s[:],
                                op=ALU.add,
                            )
                        if after_block is not None:
                            after_block(b)

            # ---------------- phase A (+ overlapped B) ----------------
            t2slab_tiles = []
            slab_done = [False] * NSLAB

            def maybe_fire_slabs(done_b):
                """Store any slab whose blocks are all done (spreads the
                t2loc HBM writes through phase A)."""
                for k, (b0_, nb_) in enumerate(slabs):
                    if slab_done[k] or done_b < b0_ + nb_ - 1:
                        continue
                    slab_done[k] = True
                    st = t2slab_tiles[k]
                    nc.sync.dma_start(
                        out=t2loc[b0_ * BN : (b0_ + nb_) * BN, :].rearrange(
                            "(b p) c -> p b c", p=128
                        ),
                        in_=st[:].rearrange("p (b c) -> p b c", b=nb_),
                    )

            phases = cfg.get("PHASES", "A,B,C").split(",")
            if "A" in phases:
                for k, (b0_, nb_) in enumerate(slabs):
                    st = cstp.tile([128, nb_ * FW], F16, name=f"t2st{k}")
                    t2slab_tiles.append(st)

                layer_pass(
                    0,
                    layouts[0],
                    idxg1_d,
                    dst1_d,
                    lambda s: x_d[s * CH : (s + 1) * CH, :],
                    lambda b: xloc_s[:, b * FW : b * FW + F],
                    t2slab_tiles,
                    after_block=maybe_fire_slabs,
                )
                maybe_fire_slabs(NB - 1)

            # ------- L2 gather prefetch: write descriptors during B -------
            PREFETCH = cfg.get("PREFETCH", 0)
            pre_emitted = {}
            pre_queues = set()
            if "C" in phases and PREFETCH > 0:
                for gi in range(min(PREFETCH, len(layouts[0]["groups"]))):
                    grp = layouts[0]["groups"][gi]
                    xg, dst_t, used_q = emit_group_gathers(
                        grp,
                        idxg1_d,
                        dst1_d,
                        lambda s: t2full[s * CH : (s + 1) * CH, :],
                        prepare=True,
                        sem=f"l2pre{gi}",
                    )
                    pre_emitted[gi] = (xg, dst_t)
                    pre_queues.update(used_q)

            # ---------------- phase B: AllGather ----------------
            if "B" in phases:
                if NCORES > 1:
                    nc.gpsimd.collective_compute(
                        "AllGather",
                        ALU.bypass,
                        replica_groups=[list(range(NCORES))],
                        ins=[t2loc[:, :]],
                        outs=[t2full[:, :]],
                    )
                else:
                    nc.sync.dma_start(out=t2full[:, :], in_=t2loc[:, :])
                for q in sorted(pre_queues):
                    nc.gpsimd.trigger_dma(count=None, queue_num=q)

            # ---------------- phase C ----------------
            if "C" in phases:
                out_done = [False] * NSLAB

                def maybe_store_out(done_b):
                    for k, (b0_, nb_) in enumerate(slabs):
                        if out_done[k] or done_b < b0_ + nb_ - 1:
                            continue
                        out_done[k] = True
                        nc.sync.dma_start(
                            out=out_d[
                                b0_ * BN : (b0_ + nb_) * BN, :
                            ].rearrange("(b p) c -> p b c", p=128),
                            in_=outst_k[k][:].rearrange(
                                "p (b c) -> p b c", b=nb_
                            ),
                        )

                def srcloc2(b):
                    k, boff = slab_of(b)
                    return t2slab_tiles[k][:, boff * FW : boff * FW + F]

                layer_pass(
                    1,
                    layouts[0],
                    idxg1_d,
                    dst1_d,
                    lambda s: t2full[s * CH : (s + 1) * CH, :],
                    srcloc2,
                    after_block=maybe_store_out,
                    pre_emitted=pre_emitted,
                )
                maybe_store_out(NB - 1)

    nc.compile()
    return nc


# ====================== host-side preprocessing ======================


def _build_layout(
    src_bucket, src_idx, blk, within, core, NCORES, NB, GB, order_key_extra=None
):
    """Bucket edges by (core, block, bucket), pack densely per (group,
    bucket) region with max-over-cores run lengths.

    src_bucket: per-edge bucket id in [0, NCHUNK)
    src_idx: per-edge row index within its bucket's table
    Returns (G_cols, groups, per-core (idxg, dstlocg)).
    """
    E = len(src_bucket)
    key = ((core * NB + blk) * NCHUNK + src_bucket).astype(np.int64)
    order = np.argsort(key, kind="stable")
    key_o = key[order]
    idx_o = src_idx[order]
    within_o = within[order]
    blk_o = blk[order]

    counts = np.bincount(key_o, minlength=NCORES * NB * NCHUNK).reshape(
        NCORES, NB, NCHUNK
    )
    cnt_max = counts.max(axis=0)

    ngroups = -(-NB // GB)
    groups = []
    col = 0
    run_off = np.zeros((NB, NCHUNK), dtype=np.int64)
    reg_col0 = np.zeros((ngroups, NCHUNK), dtype=np.int64)
    for g in range(ngroups):
        bs = list(range(g * GB, min((g + 1) * GB, NB)))
        grp = {"col0": col, "blocks": [], "k_gs": [], "s_col0": [], "g": g}
        rng_by_block = {b: [] for b in bs}
        for s in range(NCHUNK):
            grp["s_col0"].append(col)
            reg_col0[g, s] = col
            off = 0
            for b in bs:
                c = int(cnt_max[b, s])
                run_off[b, s] = off
                if c > 0:
                    lo = col + off // 128
                    hi = col + -(-(off + c) // 128)
                    rng_by_block[b].append((lo, hi, off % 128 != 0))
                off += c
            k_gs = -(-off // 128)
            col += k_gs
            grp["k_gs"].append(k_gs)
        grp["ncols"] = col - grp["col0"]
        for b in bs:
            grp["blocks"].append(
                {"b": b, "bg": b - g * GB, "ranges": rng_by_block[b]}
            )
        groups.append(grp)
    G_cols = col

    run_start = np.zeros(NCORES * NB * NCHUNK + 1, dtype=np.int64)
    np.cumsum(counts.reshape(-1), out=run_start[1:])
    total = len(key_o)
    j_in_run = np.arange(total) - run_start[key_o]

    per_core = []
    BN = 128
    for cidx in range(NCORES):
        lo = run_start[cidx * NB * NCHUNK]
        hi = run_start[(cidx + 1) * NB * NCHUNK]
        sl = slice(lo, hi)
        k_loc = key_o[sl] - cidx * NB * NCHUNK
        b_loc = k_loc // NCHUNK
        s_loc = k_loc % NCHUNK
        j_loc = j_in_run[sl]
        g_loc = b_loc // GB
        pos = run_off[b_loc, s_loc] + j_loc
        col_abs = reg_col0[g_loc, s_loc] + pos // 128
        p_loc = pos % 128

        # dst value = within + BN if the edge sits in a column whose first
        # slot belongs to the previous block's run (shared boundary column)
        shared = (run_off[b_loc, s_loc] % 128 != 0) & (
            pos // 128 == run_off[b_loc, s_loc] // 128
        )
        dstlocg = np.full((128, G_cols), -1.0, dtype=np.float32)
        dstlocg[p_loc, col_abs] = within_o[sl] + BN * shared

        idxval = idx_o[sl].astype(np.int16)
        idxg = np.zeros((128, 8 * G_cols), dtype=np.int16)
        rowi = (pos % 16).astype(np.int64)
        coli = 8 * reg_col0[g_loc, s_loc] + pos // 16
        idxg[rowi, coli] = idxval
        idxg16 = idxg[:16]
        for kk in range(1, 8):
            idxg[16 * kk : 16 * (kk + 1)] = idxg16
        per_core.append((idxg, dstlocg.astype(np.float16)))

    layout = {"G_cols": G_cols, "groups": groups}
    return layout, per_core


def prep(
    x, edge_index, W1, b1, W2, b2, NCORES=8, BN=128, GB=6, OHSLAB=8, NSLAB=4
):
    """Partition/pad inputs. Returns (cfg, layouts, in_maps)."""
    N, F = x.shape
    H = W1.shape[1]
    C = W2.shape[1]
    assert N % NCORES == 0
    NLOC = N // NCORES
    NB = -(-NLOC // BN)
    NLOCP = NB * BN
    NPAD = NCORES * NLOCP
    assert NPAD % NCHUNK == 0
    CH = NPAD // NCHUNK
    assert CH <= 32768, "chunk exceeds int16 index range"

    src = np.asarray(edge_index[0], dtype=np.int64)
    dst = np.asarray(edge_index[1], dtype=np.int64)

    deg = np.bincount(dst, minlength=N).astype(np.float64) + 1.0
    dinv = (1.0 / np.sqrt(deg)).astype(np.float32)

    # pre-scale x by dinv (source-side factor of Ahat)
    xs = np.asarray(x, dtype=np.float32) * dinv[:, None]
    x_pad = np.zeros((NPAD, FW), dtype=np.float16)
    xv = x_pad.reshape(NCORES, NLOCP, FW)
    xv[:, :NLOC, :F] = xs.reshape(NCORES, NLOC, F).astype(np.float16)
    src_pad = src + (NLOCP - NLOC) * (src // NLOC)

    core = dst // NLOC
    dstloc = dst - core * NLOC
    blk = dstloc // BN
    within = (dstloc % BN).astype(np.float32)

    # slabs of blocks (staged t2 stores during phase A)
    bps = -(-NB // NSLAB)
    slabs = []
    b0 = 0
    while b0 < NB:
        nb_ = min(bps, NB - b0)
        slabs.append((b0, nb_))
        b0 += nb_

    # shared layout for both layers: bucket = src table quarter
    lay1, pc1 = _build_layout(
        src_pad // CH, src_pad % CH, blk, within, core, NCORES, NB, GB
    )
    layouts = [lay1]
    cfg = dict(
        NPAD=NPAD,
        NLOCP=NLOCP,
        NLOC=NLOC,
        NB=NB,
        BN=BN,
        F=F,
        H=H,
        C=C,
        NCORES=NCORES,
        GB=GB,
        OHSLAB=OHSLAB,
        slabs=slabs,
    )

    # iota: first BN columns are 128..255 (leading shared column), the
    # remaining OHSLAB copies are 0..127
    iota = np.zeros((128, (OHSLAB + 1) * BN), dtype=np.float16)
    iota[:, :BN] = (BN + np.arange(BN, dtype=np.float16))[None, :]
    iota[:, BN:] = np.tile(np.arange(BN, dtype=np.float16), OHSLAB)[None, :]
    ident = np.eye(128, dtype=np.float16)
    b1rep = np.broadcast_to(
        np.asarray(b1, dtype=np.float32)[None, :], (128, H)
    ).copy()
    b2rep = np.broadcast_to(
        np.asarray(b2, dtype=np.float32)[None, :], (128, C)
    ).copy()

    in_maps = []
    for cidx in range(NCORES):
        dinvb = np.zeros((128, NB), dtype=np.float32)
        nodes = np.arange(NLOC)
        dinvb[nodes % BN, nodes // BN] = dinv[
            cidx * NLOC : (cidx + 1) * NLOC
        ]
        in_maps.append(
            {
                "x_pad": x_pad,
                "x_loc": np.ascontiguousarray(xv[cidx]),
                "idxg1": pc1[cidx][0],
                "dst1": pc1[cidx][1],
                "dinvb": dinvb,
                "W1": np.asarray(W1, dtype=np.float16),
                "b1rep": b1rep,
                "W2": np.asarray(W2, dtype=np.float16),
                "b2rep": b2rep,
                "iota": iota,
                "ident": ident,
            }
        )

    return cfg, layouts, in_maps


def postprocess(cfg, results):
    NLOC = cfg["NLOC"]
    outs = [r["out"][:NLOC] for r in results]
    return np.concatenate(outs, axis=0)


# ====================== harness entrypoint ======================

_CACHE = {}
LAST_EXEC_NS = None


def kernel(**inputs):
    """Full-input GCN2 forward on 8 TRN2 NeuronCores.

    Shards nodes across the 8 cores (edges partitioned by destination),
    runs the Bass kernel via run_bass_kernel_spmd, gathers the output.
    """
    global LAST_EXEC_NS
    import os

    from concourse.bass_utils import run_bass_kernel_spmd

    x = np.asarray(inputs["x"], dtype=np.float32)
    edge_index = np.asarray(inputs["edge_index"])
    W1 = np.asarray(inputs["W1"], dtype=np.float32)
    b1 = np.asarray(inputs["b1"], dtype=np.float32)
    W2 = np.asarray(inputs["W2"], dtype=np.float32)
    b2 = np.asarray(inputs["b2"], dtype=np.float32)

    NCORES = 8
    cfg, layouts, in_maps = prep(
        x, edge_index, W1, b1, W2, b2, NCORES=NCORES
    )
    key = (
        x.shape,
        edge_index.shape,
        layouts[0]["G_cols"],
        tuple(tuple(g["k_gs"]) for g in layouts[0]["groups"]),
    )
    nc = _CACHE.get(key)
    if nc is None:
        nc = build_gcn_nc(cfg, layouts)
        _CACHE[key] = nc

    trace = os.environ.get("GCN_TRACE", "0") == "1"
    res = run_bass_kernel_spmd(
        nc, in_maps, core_ids=list(range(NCORES)), trace=trace
    )
    LAST_EXEC_NS = res.exec_time_ns
    out = postprocess(cfg, res.results)
    return out.astype(np.float32)
